# revision 9
# baseline (speedup 1.0000x reference)
"""AdditiveAttention (d2l-style) Trainium2 kernel, SPMD over 8 NeuronCores.

Problem shapes: B=16, Q=64, K=1024, DQ=DK=256, H=128, DV=256.

Sharding: data-parallel over the QUERY axis (8 queries per core), with every
core processing all 16 batches. This keeps the single SPMD instruction
stream identical across cores while allowing the graph (built at runtime
from the actual valid_lens values) to skip fully-invalid key tiles — a
large win since valid_lens average ~K/2.

Per-core pipeline:
  - host packs keys^T (valid 128-tiles only) and [values | ones] as fp16
  - PE: kproj^T = Wk^T @ keys^T per tile; qproj^T likewise
  - ACT: feat = tanh(kproj^T + qproj_col) with qproj as per-partition bias
  - PE: scores col = feat_tile^T @ wv  (into one [128k, T*8] PSUM tile,
        already transposed for the attention@V matmul; PE outputs must
        start at partition 0/32/64, so row-major scores are not an option)
  - ACT: e = exp(scores)  (no max-subtract needed; |scores| <~ sum|wv|)
  - PE: [out | denom] = e_task^T @ [V | 1] accumulated over valid tiles,
        partial-tile rows zeroed in e first
  - DVE: out = out * (1/denom) ; DMA to DRAM
"""

import sys

sys.path.insert(0, "/opt/trn_rl_repo")

from contextlib import ExitStack

import numpy as np

import concourse.bass as bass  # noqa: F401  (bass must import before tile)
import concourse.tile as tile
from concourse import bacc, masks, mybir
from concourse.bass_utils import run_bass_kernel_spmd

B, Q, KLEN, D, H, DV = 16, 64, 1024, 256, 128, 256
N_CORES = 8
QS = Q // N_CORES  # queries per core = 8
KT = 128  # key tile size

F16 = mybir.dt.float16
F32 = mybir.dt.float32


def _build_graph(tiles, vls):
    """tiles[b] = number of valid 128-key tiles for batch b; vls[b] = valid_lens[b]."""
    T = int(sum(tiles))
    offs = np.concatenate([[0], np.cumsum(tiles)]).astype(int)

    nc = bacc.Bacc("TRN2", target_bir_lowering=False, debug=False, num_devices=N_CORES)
    kT_d = nc.dram_tensor("kT", [T, 2, KT, KT], F16, kind="ExternalInput").ap()
    v1_d = nc.dram_tensor("v1", [T, KT, DV + 1], F16, kind="ExternalInput").ap()
    qT_d = nc.dram_tensor("qT", [2, KT, B * QS], F16, kind="ExternalInput").ap()
    wq_d = nc.dram_tensor("wqc", [2, KT, H], F16, kind="ExternalInput").ap()
    wk_d = nc.dram_tensor("wkc", [2, KT, H], F16, kind="ExternalInput").ap()
    wv_d = nc.dram_tensor("wv", [H, 1], F16, kind="ExternalInput").ap()
    out_d = nc.dram_tensor("out", [B * QS, DV], F32, kind="ExternalOutput").ap()

    with tile.TileContext(nc) as tc, ExitStack() as ctx:
        const = ctx.enter_context(tc.tile_pool(name="const", bufs=1))
        spool = ctx.enter_context(tc.tile_pool(name="s_psum", bufs=1, space="PSUM"))
        epool = ctx.enter_context(tc.tile_pool(name="e_sb", bufs=1))
        vpool = ctx.enter_context(tc.tile_pool(name="vals", bufs=T))
        fin = ctx.enter_context(tc.tile_pool(name="fin", bufs=1))

        # constants
        wq_sb = [const.tile([KT, H], F16, tag=f"wq{c}", name=f"wq{c}") for c in range(2)]
        wk_sb = [const.tile([KT, H], F16, tag=f"wk{c}", name=f"wk{c}") for c in range(2)]
        qT_sb = [const.tile([KT, B * QS], F16, tag=f"qt{c}", name=f"qt{c}") for c in range(2)]
        for c in range(2):
            nc.sync.dma_start(wq_sb[c][:], wq_d[c])
            nc.sync.dma_start(wk_sb[c][:], wk_d[c])
            nc.sync.dma_start(qT_sb[c][:], qT_d[c])
        wv_sb = const.tile([H, 1], F16, tag="wv", name="wv_sb")
        nc.sync.dma_start(wv_sb[:], wv_d[:])
        qproj_sb = const.tile([H, B * QS], F32, tag="qproj", name="qproj_sb")

        # transposed scores: partition = key-in-tile, free = task*QS + j
        S = spool.tile([KT, T * QS], F32)

        # ---- phase 0: query projection ----
        with tc.tile_pool(name="qp_psum", bufs=1, space="PSUM") as qpp:
            qp = qpp.tile([H, B * QS], F32)
            for c in range(2):
                nc.tensor.matmul(
                    qp[:], wq_sb[c][:], qT_sb[c][:], start=(c == 0), stop=(c == 1)
                )
            nc.vector.tensor_copy(qproj_sb[:], qp[:])

        vals_sb = []

        # ---- phase 1: kproj, tanh features, score matvecs ----
        with (
            tc.tile_pool(name="kt_sb", bufs=6) as ktp,
            tc.tile_pool(name="kp_psum", bufs=2, space="PSUM") as kpp,
            tc.tile_pool(name="kproj_sb", bufs=3) as kjp,
            tc.tile_pool(name="feat", bufs=4) as fp,
        ):
            kprojs = {}

            def emit_kproj(b):
                nt = int(tiles[b])
                vlpad = nt * KT
                kp = kpp.tile([H, KLEN], F32, tag="kp", name="kp")
                for t in range(nt):
                    task = int(offs[b]) + t
                    # prefetch the [V | 1] tile for phase 2
                    vt = vpool.tile([KT, DV + 1], F16, tag="v1", name="vt")
                    nc.sync.dma_start(vt[:], v1_d[task])
                    vals_sb.append(vt)
                    k0 = ktp.tile([KT, KT], F16, tag="k0", name="k0")
                    k1 = ktp.tile([KT, KT], F16, tag="k1", name="k1")
                    nc.sync.dma_start(k0[:], kT_d[task, 0])
                    nc.sync.dma_start(k1[:], kT_d[task, 1])
                    sl = kp[:, t * KT : (t + 1) * KT]
                    nc.tensor.matmul(sl, wk_sb[0][:], k0[:], start=True, stop=False)
                    nc.tensor.matmul(sl, wk_sb[1][:], k1[:], start=False, stop=True)
                kproj = kjp.tile([H, KLEN], F16, tag="kproj", name="kproj")
                nc.vector.tensor_copy(kproj[:, :vlpad], kp[:, :vlpad])
                kprojs[b] = kproj

            # kproj is computed one batch ahead so PE's in-order stream can
            # project batch b+1 while ACT runs batch b's tanh instructions.
            emit_kproj(0)
            for b in range(B):
                if b + 1 < B:
                    emit_kproj(b + 1)
                nt = int(tiles[b])
                vlpad = nt * KT
                kproj = kprojs.pop(b)
                for j in range(QS):
                    bq = b * QS + j
                    feat = fp.tile([H, KLEN], F16, tag="feat", name="feat")
                    nc.scalar.activation(
                        feat[:, :vlpad],
                        kproj[:, :vlpad],
                        mybir.ActivationFunctionType.Tanh,
                        bias=qproj_sb[:, bq : bq + 1],
                    )
                    for t in range(nt):
                        gcol = (int(offs[b]) + t) * QS + j
                        nc.tensor.matmul(
                            S[:, gcol : gcol + 1],
                            feat[:, t * KT : (t + 1) * KT],
                            wv_sb[:],
                            start=True,
                            stop=True,
                        )

        # ---- phase 2: exp, attention @ [V | 1] ----
        # invalid key rows of each batch's final partial tile are handled by
        # host-side zeroing of the corresponding [V | 1] rows, so e needs no
        # masking here.
        e = epool.tile([KT, T * QS], F16)
        nc.scalar.activation(e[:, :], S[:, :], mybir.ActivationFunctionType.Exp)

        with (
            tc.tile_pool(name="o_psum", bufs=3, space="PSUM") as op,
        ):
            for b in range(B):
                nt = int(tiles[b])
                Ob = op.tile([QS, DV + 1], F32, tag="ob", name="Ob")
                for t in range(nt):
                    task = int(offs[b]) + t
                    nc.tensor.matmul(
                        Ob[:],
                        e[:, task * QS : (task + 1) * QS],
                        vals_sb[task][:],
                        start=(t == 0),
                        stop=(t == nt - 1),
                    )
                recip = fin.tile([QS, 1], F32, tag="recip", name="recip", bufs=2)
                nc.vector.reciprocal(recip[:], Ob[:, DV : DV + 1])
                outf = fin.tile([QS, DV], F32, tag="outf", name="outf", bufs=3)
                nc.vector.tensor_scalar_mul(outf[:], Ob[:, :DV], recip[:])
                nc.sync.dma_start(out_d[b * QS : (b + 1) * QS, :], outf[:])

    nc.compile()
    return nc


def kernel(queries, keys, values, valid_lens, Wq, Wk, wv):
    queries = np.asarray(queries, dtype=np.float32)
    keys = np.asarray(keys, dtype=np.float32)
    values = np.asarray(values, dtype=np.float32)
    vl = np.asarray(valid_lens).astype(np.int64)
    Wq = np.asarray(Wq, dtype=np.float32)
    Wk = np.asarray(Wk, dtype=np.float32)
    wv = np.asarray(wv, dtype=np.float32)

    tiles = np.maximum(1, -(-vl // KT))  # ceil, >=1
    T = int(tiles.sum())

    # packed keys^T: [T, 2, 128, 128] fp16 (d-chunk, d, k)
    kT_pack = np.empty((T, 2, KT, KT), dtype=np.float16)
    v1_pack = np.empty((T, KT, DV + 1), dtype=np.float16)
    v1_pack[:, :, DV] = 1.0
    i = 0
    for b in range(B):
        nt = int(tiles[b])
        for t in range(nt):
            ksl = keys[b, t * KT : (t + 1) * KT, :]  # [128, 256]
            kT_pack[i] = ksl.T.reshape(2, KT, KT).astype(np.float16)
            v1_pack[i, :, :DV] = values[b, t * KT : (t + 1) * KT, :]
            if t == nt - 1:
                rows = int(vl[b]) - t * KT
                v1_pack[i, rows:, :] = 0.0  # mask invalid keys via V and ones col
            i += 1

    wqc = Wq.reshape(2, KT, H).astype(np.float16)
    wkc = Wk.reshape(2, KT, H).astype(np.float16)
    wv_c = wv.reshape(H, 1).astype(np.float16)

    nc = _build_graph(tiles, vl)

    in_maps = []
    for c in range(N_CORES):
        qc = queries[:, c * QS : (c + 1) * QS, :].reshape(B * QS, D)
        qT = np.ascontiguousarray(qc.T).reshape(2, KT, B * QS).astype(np.float16)
        in_maps.append(
            {
                "kT": kT_pack,
                "v1": v1_pack,
                "qT": qT,
                "wqc": wqc,
                "wkc": wkc,
                "wv": wv_c,
            }
        )

    res = run_bass_kernel_spmd(nc, in_maps, core_ids=list(range(N_CORES)))

    out = np.empty((B, Q, DV), dtype=np.float32)
    for c in range(N_CORES):
        out[:, c * QS : (c + 1) * QS, :] = res.results[c]["out"].reshape(B, QS, DV)
    return out


# revision 10
# speedup vs baseline: 1.3445x; 1.3445x over previous
"""AdditiveAttention (d2l-style) Trainium2 kernel, SPMD over 8 NeuronCores.

Problem shapes: B=16, Q=64, K=1024, DQ=DK=256, H=128, DV=256.

Sharding: data-parallel over the QUERY axis (8 queries per core), with every
core processing all 16 batches. This keeps the single SPMD instruction
stream identical across cores while allowing the graph (built at runtime
from the actual valid_lens values) to skip fully-invalid key tiles — a
large win since valid_lens average ~K/2.

Per-core pipeline (batches processed smallest-first, software-pipelined):
  - host packs keys^T (valid 128-tiles only, partition-major so each batch
    is ONE contiguous DMA) and [values | ones] likewise, both fp16
  - PE: kproj^T = Wk^T @ keys^T per tile; qproj^T likewise
  - ACT: feat = tanh(kproj^T + qproj_col) with qproj as per-partition bias
  - PE: scores col = feat_tile^T @ wv into one [128k, T*8] PSUM tile
        (transposed scores; PE outputs must start at partition 0/32/64)
  - ACT: e = exp(scores) per group of batches (no max-subtract needed;
        |scores| <= sum|wv| ~ 9, exp stays in fp16 range)
  - PE: [out | denom] = e_task^T @ [V | 1] accumulated over valid tiles;
        invalid tail rows of the last tile are zeroed in V on the host
  - DVE: out = out * (1/denom) ; DMA out, all per-batch (no serial tail)
"""

import sys

sys.path.insert(0, "/opt/trn_rl_repo")

from contextlib import ExitStack

import numpy as np

import concourse.bass as bass  # noqa: F401  (bass must import before tile)
import concourse.tile as tile
from concourse import bacc, mybir
from concourse.bass_utils import run_bass_kernel_spmd

B, Q, KLEN, D, H, DV = 16, 64, 1024, 256, 128, 256
N_CORES = 8
QS = Q // N_CORES  # queries per core = 8
KT = 128  # key tile size
EXP_GROUP = 4  # batches per exp instruction

F16 = mybir.dt.float16
F32 = mybir.dt.float32


def _build_graph(order, tiles, vls):
    """order: batch processing order; tiles[b]: valid 128-key tiles; vls[b]: valid_lens."""
    ntiles = [int(tiles[b]) for b in order]
    T = int(sum(ntiles))
    # offs[i] = first task index of the i-th processed batch
    offs = np.concatenate([[0], np.cumsum(ntiles)]).astype(int)

    nc = bacc.Bacc("TRN2", target_bir_lowering=False, debug=False, num_devices=N_CORES)
    # per-batch contiguous packs, partition-major: kT[i] is [128, nt_i*256]
    kT_d = nc.dram_tensor("kT", [KT, T * 2 * KT], F16, kind="ExternalInput").ap()
    v1_d = nc.dram_tensor("v1", [KT, T * (DV + 1)], F16, kind="ExternalInput").ap()
    qT_d = nc.dram_tensor("qT", [2, KT, B * QS], F16, kind="ExternalInput").ap()
    wq_d = nc.dram_tensor("wqc", [2, KT, H], F16, kind="ExternalInput").ap()
    wk_d = nc.dram_tensor("wkc", [2, KT, H], F16, kind="ExternalInput").ap()
    wv_d = nc.dram_tensor("wv", [H, 1], F16, kind="ExternalInput").ap()
    out_d = nc.dram_tensor("out", [B * QS, DV], F32, kind="ExternalOutput").ap()

    with tile.TileContext(nc) as tc, ExitStack() as ctx:
        const = ctx.enter_context(tc.tile_pool(name="const", bufs=1))
        spool = ctx.enter_context(tc.tile_pool(name="s_psum", bufs=1, space="PSUM"))
        epool = ctx.enter_context(tc.tile_pool(name="e_sb", bufs=2))
        vpool = ctx.enter_context(tc.tile_pool(name="vals", bufs=B))
        fin = ctx.enter_context(tc.tile_pool(name="fin", bufs=1))
        opool = ctx.enter_context(tc.tile_pool(name="o_psum", bufs=2, space="PSUM"))

        # constants
        wq_sb = [const.tile([KT, H], F16, tag=f"wq{c}", name=f"wq{c}") for c in range(2)]
        wk_sb = [const.tile([KT, H], F16, tag=f"wk{c}", name=f"wk{c}") for c in range(2)]
        qT_sb = [
            const.tile([KT, B * QS], F16, tag=f"qt{c}", name=f"qt{c}") for c in range(2)
        ]
        for c in range(2):
            nc.sync.dma_start(wq_sb[c][:], wq_d[c])
            nc.sync.dma_start(wk_sb[c][:], wk_d[c])
            nc.sync.dma_start(qT_sb[c][:], qT_d[c])
        wv_sb = const.tile([H, 1], F16, tag="wv", name="wv_sb")
        nc.sync.dma_start(wv_sb[:], wv_d[:])
        qproj_sb = const.tile([H, B * QS], F32, tag="qproj", name="qproj_sb")

        # transposed scores: partition = key-in-tile, free = task*QS + j
        S = spool.tile([KT, T * QS], F32)

        # ---- phase 0: query projection ----
        with tc.tile_pool(name="qp_psum", bufs=1, space="PSUM") as qpp:
            qp = qpp.tile([H, B * QS], F32)
            for c in range(2):
                nc.tensor.matmul(
                    qp[:], wq_sb[c][:], qT_sb[c][:], start=(c == 0), stop=(c == 1)
                )
            nc.vector.tensor_copy(qproj_sb[:], qp[:])

        vals_sb = {}  # processed-batch index -> [128, nt*(DV+1)] tile

        with (
            tc.tile_pool(name="kt_sb", bufs=3) as ktp,
            tc.tile_pool(name="kp_psum", bufs=2, space="PSUM") as kpp,
            tc.tile_pool(name="kproj_sb", bufs=3) as kjp,
            tc.tile_pool(name="feat", bufs=4) as fp,
        ):
            kprojs = {}

            def emit_kproj(i):
                nt = ntiles[i]
                vlpad = nt * KT
                o = int(offs[i])
                # one contiguous DMA for the whole batch's keys^T and [V|1]
                vt = vpool.tile([KT, 8 * (DV + 1)], F16, tag="v1", name="vt")
                nc.sync.dma_start(
                    vt[:, : nt * (DV + 1)],
                    v1_d[:, o * (DV + 1) : (o + nt) * (DV + 1)],
                )
                vals_sb[i] = vt
                kt = ktp.tile([KT, 8 * 2 * KT], F16, tag="kt", name="kt")
                nc.sync.dma_start(
                    kt[:, : nt * 2 * KT], kT_d[:, o * 2 * KT : (o + nt) * 2 * KT]
                )
                kp = kpp.tile([H, KLEN], F32, tag="kp", name="kp")
                for t in range(nt):
                    sl = kp[:, t * KT : (t + 1) * KT]
                    k0 = kt[:, (2 * t) * KT : (2 * t + 1) * KT]
                    k1 = kt[:, (2 * t + 1) * KT : (2 * t + 2) * KT]
                    nc.tensor.matmul(sl, wk_sb[0][:], k0, start=True, stop=False)
                    nc.tensor.matmul(sl, wk_sb[1][:], k1, start=False, stop=True)
                kproj = kjp.tile([H, KLEN], F16, tag="kproj", name="kproj")
                nc.vector.tensor_copy(kproj[:, :vlpad], kp[:, :vlpad])
                kprojs[i] = kproj

            def emit_exp_and_av(i0, i1):
                # exp over the column range of processed batches [i0, i1)
                c0, c1 = int(offs[i0]) * QS, int(offs[i1]) * QS
                e = epool.tile([KT, EXP_GROUP * 8 * QS], F16, tag="e", name="e")
                nc.scalar.activation(
                    e[:, : c1 - c0], S[:, c0:c1], mybir.ActivationFunctionType.Exp
                )
                for i in range(i0, i1):
                    b = order[i]
                    nt = ntiles[i]
                    Ob = opool.tile([QS, DV + 1], F32, tag="ob", name="Ob")
                    for t in range(nt):
                        ec = (int(offs[i]) + t) * QS - c0
                        nc.tensor.matmul(
                            Ob[:],
                            e[:, ec : ec + QS],
                            vals_sb[i][:, t * (DV + 1) : (t + 1) * (DV + 1)],
                            start=(t == 0),
                            stop=(t == nt - 1),
                        )
                    vals_sb[i] = None
                    recip = fin.tile([QS, 1], F32, tag="recip", name="recip", bufs=2)
                    nc.vector.reciprocal(recip[:], Ob[:, DV : DV + 1])
                    outf = fin.tile([QS, DV], F32, tag="outf", name="outf", bufs=3)
                    nc.vector.tensor_scalar_mul(outf[:], Ob[:, :DV], recip[:])
                    nc.sync.dma_start(out_d[b * QS : (b + 1) * QS, :], outf[:])

            # kproj is computed one batch ahead so PE's in-order stream can
            # project batch i+1 while ACT runs batch i's tanh instructions.
            emit_kproj(0)
            for i in range(B):
                if i + 1 < B:
                    emit_kproj(i + 1)
                nt = ntiles[i]
                vlpad = nt * KT
                kproj = kprojs.pop(i)
                for j in range(QS):
                    bq = order[i] * QS + j
                    feat = fp.tile([H, KLEN], F16, tag="feat", name="feat")
                    nc.scalar.activation(
                        feat[:, :vlpad],
                        kproj[:, :vlpad],
                        mybir.ActivationFunctionType.Tanh,
                        bias=qproj_sb[:, bq : bq + 1],
                    )
                    for t in range(nt):
                        gcol = (int(offs[i]) + t) * QS + j
                        nc.tensor.matmul(
                            S[:, gcol : gcol + 1],
                            feat[:, t * KT : (t + 1) * KT],
                            wv_sb[:],
                            start=True,
                            stop=True,
                        )
                if i % EXP_GROUP == EXP_GROUP - 1:
                    emit_exp_and_av(i - EXP_GROUP + 1, i + 1)

    nc.compile()
    return nc


def kernel(queries, keys, values, valid_lens, Wq, Wk, wv):
    queries = np.asarray(queries, dtype=np.float32)
    keys = np.asarray(keys, dtype=np.float32)
    values = np.asarray(values, dtype=np.float32)
    vl = np.asarray(valid_lens).astype(np.int64)
    Wq = np.asarray(Wq, dtype=np.float32)
    Wk = np.asarray(Wk, dtype=np.float32)
    wv = np.asarray(wv, dtype=np.float32)

    tiles = np.maximum(1, -(-vl // KT))  # ceil, >=1
    order = np.argsort(tiles, kind="stable")  # smallest batches first
    T = int(tiles.sum())

    # packed keys^T, partition-major: column block per task of [2*128] (d-chunks)
    kT_pack = np.empty((KT, T, 2, KT), dtype=np.float16)
    v1_pack = np.empty((KT, T, DV + 1), dtype=np.float16)
    v1_pack[:, :, DV] = 1.0
    i = 0
    for i_proc in range(B):
        b = int(order[i_proc])
        nt = int(tiles[b])
        for t in range(nt):
            ksl = keys[b, t * KT : (t + 1) * KT, :]  # [128 k, 256 d]
            # kT_pack[p, i, c, :] = keys[b, t*KT + (c*128..), p]^T chunks
            kT_pack[:, i, 0, :] = ksl[:, :KT].T
            kT_pack[:, i, 1, :] = ksl[:, KT:].T
            v1_pack[:, i, :DV] = values[b, t * KT : (t + 1) * KT, :]
            if t == nt - 1:
                rows = int(vl[b]) - t * KT
                v1_pack[rows:, i, :] = 0.0  # mask invalid keys via V and ones col
            i += 1
    kT_pack = kT_pack.reshape(KT, T * 2 * KT)
    v1_pack = v1_pack.reshape(KT, T * (DV + 1))

    wqc = Wq.reshape(2, KT, H).astype(np.float16)
    wkc = Wk.reshape(2, KT, H).astype(np.float16)
    wv_c = wv.reshape(H, 1).astype(np.float16)

    nc = _build_graph(order, tiles, vl)

    in_maps = []
    for c in range(N_CORES):
        qc = queries[:, c * QS : (c + 1) * QS, :].reshape(B * QS, D)
        qT = np.ascontiguousarray(qc.T).reshape(2, KT, B * QS).astype(np.float16)
        in_maps.append(
            {
                "kT": kT_pack,
                "v1": v1_pack,
                "qT": qT,
                "wqc": wqc,
                "wkc": wkc,
                "wv": wv_c,
            }
        )

    res = run_bass_kernel_spmd(nc, in_maps, core_ids=list(range(N_CORES)))

    out = np.empty((B, Q, DV), dtype=np.float32)
    for c in range(N_CORES):
        out[:, c * QS : (c + 1) * QS, :] = res.results[c]["out"].reshape(B, QS, DV)
    return out


# revision 11
# speedup vs baseline: 1.3941x; 1.0369x over previous
"""AdditiveAttention (d2l-style) Trainium2 kernel, SPMD over 8 NeuronCores.

Problem shapes: B=16, Q=64, K=1024, DQ=DK=256, H=128, DV=256.

Sharding: data-parallel over the QUERY axis (8 queries per core), with every
core processing all 16 batches. This keeps the single SPMD instruction
stream identical across cores while allowing the graph (built at runtime
from the actual valid_lens values) to skip fully-invalid key tiles — a
large win since valid_lens average ~K/2.

Per-core pipeline (batches processed smallest-first, software-pipelined):
  - host packs keys^T (valid 128-tiles only, partition-major so each batch
    is ONE contiguous DMA) and [values | ones] likewise, both fp16
  - PE: kproj^T = Wk^T @ keys^T per tile; qproj^T likewise
  - ACT: feat = tanh(kproj^T + qproj_col) with qproj as per-partition bias
  - PE: scores col = feat_tile^T @ wv into one [128k, T*8] PSUM tile
        (transposed scores; PE outputs must start at partition 0/32/64)
  - ACT: e = exp(scores) per group of batches (no max-subtract needed;
        |scores| <= sum|wv| ~ 9, exp stays in fp16 range)
  - PE: [out | denom] = e_task^T @ [V | 1] accumulated over valid tiles;
        invalid tail rows of the last tile are zeroed in V on the host
  - DVE: out = out * (1/denom) ; DMA out, all per-batch (no serial tail)
"""

import sys

sys.path.insert(0, "/opt/trn_rl_repo")

from contextlib import ExitStack

import numpy as np

import concourse.bass as bass  # noqa: F401  (bass must import before tile)
import concourse.tile as tile
from concourse import bacc, mybir
from concourse.bass_utils import run_bass_kernel_spmd

B, Q, KLEN, D, H, DV = 16, 64, 1024, 256, 128, 256
N_CORES = 8
QS = Q // N_CORES  # queries per core = 8
KT = 128  # key tile size
EXP_GROUP = 4  # batches per exp instruction

F16 = mybir.dt.float16
F32 = mybir.dt.float32


def _build_graph(order, tiles, vls):
    """order: batch processing order; tiles[b]: valid 128-key tiles; vls[b]: valid_lens."""
    ntiles = [int(tiles[b]) for b in order]
    nvalid = [int(vls[b]) for b in order]
    T = int(sum(ntiles))
    # offs[i] = first task index of the i-th processed batch
    offs = np.concatenate([[0], np.cumsum(ntiles)]).astype(int)

    nc = bacc.Bacc("TRN2", target_bir_lowering=False, debug=False, num_devices=N_CORES)
    # per-batch contiguous packs, partition-major: kT[i] is [128, nt_i*256]
    kT_d = nc.dram_tensor("kT", [KT, T * 2 * KT], F16, kind="ExternalInput").ap()
    v1_d = nc.dram_tensor("v1", [KT, T * (DV + 1)], F16, kind="ExternalInput").ap()
    qT_d = nc.dram_tensor("qT", [2, KT, B * QS], F16, kind="ExternalInput").ap()
    wq_d = nc.dram_tensor("wqc", [2, KT, H], F16, kind="ExternalInput").ap()
    wk_d = nc.dram_tensor("wkc", [2, KT, H], F16, kind="ExternalInput").ap()
    wv_d = nc.dram_tensor("wv", [H, 1], F16, kind="ExternalInput").ap()
    out_d = nc.dram_tensor("out", [B * QS, DV], F32, kind="ExternalOutput").ap()

    with tile.TileContext(nc) as tc, ExitStack() as ctx:
        const = ctx.enter_context(tc.tile_pool(name="const", bufs=1))
        spool = ctx.enter_context(tc.tile_pool(name="s_psum", bufs=1, space="PSUM"))
        epool = ctx.enter_context(tc.tile_pool(name="e_sb", bufs=2))
        vpool = ctx.enter_context(tc.tile_pool(name="vals", bufs=5))
        fin = ctx.enter_context(tc.tile_pool(name="fin", bufs=1))
        opool = ctx.enter_context(tc.tile_pool(name="o_psum", bufs=2, space="PSUM"))

        # constants
        wq_sb = [const.tile([KT, H], F16, tag=f"wq{c}", name=f"wq{c}") for c in range(2)]
        wk_sb = [const.tile([KT, H], F16, tag=f"wk{c}", name=f"wk{c}") for c in range(2)]
        qT_sb = [
            const.tile([KT, B * QS], F16, tag=f"qt{c}", name=f"qt{c}") for c in range(2)
        ]
        for c in range(2):
            nc.sync.dma_start(wq_sb[c][:], wq_d[c])
            nc.sync.dma_start(wk_sb[c][:], wk_d[c])
            nc.sync.dma_start(qT_sb[c][:], qT_d[c])
        wv_sb = const.tile([H, 1], F16, tag="wv", name="wv_sb")
        nc.sync.dma_start(wv_sb[:], wv_d[:])
        qproj_sb = const.tile([H, B * QS], F32, tag="qproj", name="qproj_sb")

        # transposed scores: partition = key-in-tile, free = task*QS + j
        S = spool.tile([KT, T * QS], F32)
        # partial-tile columns are only written up to their valid rows;
        # zero once so exp() of the stale rows is finite (killed by V=0)
        nc.vector.memset(S[:, :], 0.0)

        # ---- phase 0: query projection ----
        with tc.tile_pool(name="qp_psum", bufs=1, space="PSUM") as qpp:
            qp = qpp.tile([H, B * QS], F32)
            for c in range(2):
                nc.tensor.matmul(
                    qp[:], wq_sb[c][:], qT_sb[c][:], start=(c == 0), stop=(c == 1)
                )
            nc.vector.tensor_copy(qproj_sb[:], qp[:])

        vals_sb = {}  # processed-batch index -> [128, nt*(DV+1)] tile

        with (
            tc.tile_pool(name="kt_sb", bufs=3) as ktp,
            tc.tile_pool(name="kp_psum", bufs=2, space="PSUM") as kpp,
            tc.tile_pool(name="kproj_sb", bufs=3) as kjp,
            tc.tile_pool(name="feat", bufs=4) as fp,
        ):
            kprojs = {}

            def emit_kproj(i):
                nt = ntiles[i]
                vlpad = nt * KT
                o = int(offs[i])
                # one contiguous DMA for the whole batch's keys^T and [V|1]
                vt = vpool.tile([KT, 8 * (DV + 1)], F16, tag="v1", name="vt")
                nc.sync.dma_start(
                    vt[:, : nt * (DV + 1)],
                    v1_d[:, o * (DV + 1) : (o + nt) * (DV + 1)],
                )
                vals_sb[i] = vt
                kt = ktp.tile([KT, 8 * 2 * KT], F16, tag="kt", name="kt")
                nc.sync.dma_start(
                    kt[:, : nt * 2 * KT], kT_d[:, o * 2 * KT : (o + nt) * 2 * KT]
                )
                vl_i = nvalid[i]
                kp = kpp.tile([H, KLEN], F32, tag="kp", name="kp")
                for t in range(nt):
                    w = min(KT, vl_i - t * KT)
                    sl = kp[:, t * KT : t * KT + w]
                    k0 = kt[:, (2 * t) * KT : (2 * t) * KT + w]
                    k1 = kt[:, (2 * t + 1) * KT : (2 * t + 1) * KT + w]
                    nc.tensor.matmul(sl, wk_sb[0][:], k0, start=True, stop=False)
                    nc.tensor.matmul(sl, wk_sb[1][:], k1, start=False, stop=True)
                kproj = kjp.tile([H, KLEN], F16, tag="kproj", name="kproj")
                nc.vector.tensor_copy(kproj[:, :vl_i], kp[:, :vl_i])
                kprojs[i] = kproj

            def emit_exp_and_av(i0, i1):
                # exp over the column range of processed batches [i0, i1)
                c0, c1 = int(offs[i0]) * QS, int(offs[i1]) * QS
                e = epool.tile([KT, EXP_GROUP * 8 * QS], F16, tag="e", name="e")
                nc.scalar.activation(
                    e[:, : c1 - c0], S[:, c0:c1], mybir.ActivationFunctionType.Exp
                )
                for i in range(i0, i1):
                    b = order[i]
                    nt = ntiles[i]
                    Ob = opool.tile([QS, DV + 1], F32, tag="ob", name="Ob")
                    for t in range(nt):
                        ec = (int(offs[i]) + t) * QS - c0
                        nc.tensor.matmul(
                            Ob[:],
                            e[:, ec : ec + QS],
                            vals_sb[i][:, t * (DV + 1) : (t + 1) * (DV + 1)],
                            start=(t == 0),
                            stop=(t == nt - 1),
                        )
                    vals_sb[i] = None
                    recip = fin.tile([QS, 1], F32, tag="recip", name="recip", bufs=2)
                    nc.vector.reciprocal(recip[:], Ob[:, DV : DV + 1])
                    outf = fin.tile([QS, DV], F32, tag="outf", name="outf", bufs=3)
                    nc.vector.tensor_scalar_mul(outf[:], Ob[:, :DV], recip[:])
                    nc.sync.dma_start(out_d[b * QS : (b + 1) * QS, :], outf[:])

            # kproj is computed one batch ahead so PE's in-order stream can
            # project batch i+1 while ACT runs batch i's tanh instructions.
            emit_kproj(0)
            for i in range(B):
                if i + 1 < B:
                    emit_kproj(i + 1)
                nt = ntiles[i]
                vl_i = nvalid[i]
                kproj = kprojs.pop(i)
                for j in range(QS):
                    bq = order[i] * QS + j
                    feat = fp.tile([H, KLEN], F16, tag="feat", name="feat")
                    nc.scalar.activation(
                        feat[:, :vl_i],
                        kproj[:, :vl_i],
                        mybir.ActivationFunctionType.Tanh,
                        bias=qproj_sb[:, bq : bq + 1],
                    )
                    for t in range(nt):
                        w = min(KT, vl_i - t * KT)
                        gcol = (int(offs[i]) + t) * QS + j
                        nc.tensor.matmul(
                            S[:w, gcol : gcol + 1],
                            feat[:, t * KT : t * KT + w],
                            wv_sb[:],
                            start=True,
                            stop=True,
                        )
                if i % EXP_GROUP == EXP_GROUP - 1:
                    emit_exp_and_av(i - EXP_GROUP + 1, i + 1)

    nc.compile()
    return nc


def kernel(queries, keys, values, valid_lens, Wq, Wk, wv):
    queries = np.asarray(queries, dtype=np.float32)
    keys = np.asarray(keys, dtype=np.float32)
    values = np.asarray(values, dtype=np.float32)
    vl = np.asarray(valid_lens).astype(np.int64)
    Wq = np.asarray(Wq, dtype=np.float32)
    Wk = np.asarray(Wk, dtype=np.float32)
    wv = np.asarray(wv, dtype=np.float32)

    tiles = np.maximum(1, -(-vl // KT))  # ceil, >=1
    asc = np.argsort(tiles, kind="stable")
    # smallest batch first (fast ramp), then descending so the last
    # processed batches (the exp/AV tail) are small again
    order = np.concatenate([asc[:1], asc[1:][::-1]])
    T = int(tiles.sum())

    # packed keys^T, partition-major: column block per task of [2*128] (d-chunks)
    kT_pack = np.empty((KT, T, 2, KT), dtype=np.float16)
    v1_pack = np.empty((KT, T, DV + 1), dtype=np.float16)
    v1_pack[:, :, DV] = 1.0
    i = 0
    for i_proc in range(B):
        b = int(order[i_proc])
        nt = int(tiles[b])
        for t in range(nt):
            ksl = keys[b, t * KT : (t + 1) * KT, :]  # [128 k, 256 d]
            # kT_pack[p, i, c, :] = keys[b, t*KT + (c*128..), p]^T chunks
            kT_pack[:, i, 0, :] = ksl[:, :KT].T
            kT_pack[:, i, 1, :] = ksl[:, KT:].T
            v1_pack[:, i, :DV] = values[b, t * KT : (t + 1) * KT, :]
            if t == nt - 1:
                rows = int(vl[b]) - t * KT
                v1_pack[rows:, i, :] = 0.0  # mask invalid keys via V and ones col
            i += 1
    kT_pack = kT_pack.reshape(KT, T * 2 * KT)
    v1_pack = v1_pack.reshape(KT, T * (DV + 1))

    wqc = Wq.reshape(2, KT, H).astype(np.float16)
    wkc = Wk.reshape(2, KT, H).astype(np.float16)
    wv_c = wv.reshape(H, 1).astype(np.float16)

    nc = _build_graph(order, tiles, vl)

    in_maps = []
    for c in range(N_CORES):
        qc = queries[:, c * QS : (c + 1) * QS, :].reshape(B * QS, D)
        qT = np.ascontiguousarray(qc.T).reshape(2, KT, B * QS).astype(np.float16)
        in_maps.append(
            {
                "kT": kT_pack,
                "v1": v1_pack,
                "qT": qT,
                "wqc": wqc,
                "wkc": wkc,
                "wv": wv_c,
            }
        )

    res = run_bass_kernel_spmd(nc, in_maps, core_ids=list(range(N_CORES)))

    out = np.empty((B, Q, DV), dtype=np.float32)
    for c in range(N_CORES):
        out[:, c * QS : (c + 1) * QS, :] = res.results[c]["out"].reshape(B, QS, DV)
    return out


# revision 12
# speedup vs baseline: 1.7670x; 1.2674x over previous
"""AdditiveAttention (d2l-style) Trainium2 kernel, SPMD over 8 NeuronCores.

Problem shapes: B=16, Q=64, K=1024, DQ=DK=256, H=128, DV=256.

Sharding: data-parallel over the QUERY axis (8 queries per core), with every
core processing all 16 batches. This keeps the single SPMD instruction
stream identical across cores while allowing the graph (built at runtime
from the actual valid_lens values) to skip fully-invalid key tiles — a
large win since valid_lens average ~K/2.

Per-core pipeline (batches processed smallest-first, software-pipelined):
  - host packs keys^T (valid 128-tiles only, partition-major so each batch
    is ONE contiguous DMA) and [values | ones] likewise, both fp16
  - PE: kproj^T = Wk^T @ keys^T per tile; qproj^T likewise
  - ACT: feat = tanh(kproj^T + qproj_col) with qproj as per-partition bias
  - PE: scores col = feat_tile^T @ wv into one [128k, T*8] PSUM tile
        (transposed scores; PE outputs must start at partition 0/32/64)
  - ACT: e = exp(scores) per group of batches (no max-subtract needed;
        |scores| <= sum|wv| ~ 9, exp stays in fp16 range)
  - PE: [out | denom] = e_task^T @ [V | 1] accumulated over valid tiles;
        invalid tail rows of the last tile are zeroed in V on the host
  - DVE: out = out * (1/denom) ; DMA out, all per-batch (no serial tail)
"""

import sys

sys.path.insert(0, "/opt/trn_rl_repo")

from contextlib import ExitStack

import numpy as np

import concourse.bass as bass  # noqa: F401  (bass must import before tile)
import concourse.tile as tile
from concourse import bacc, mybir
from concourse.bass_utils import run_bass_kernel_spmd

B, Q, KLEN, D, H, DV = 16, 64, 1024, 256, 128, 256
N_CORES = 8
QS = Q // N_CORES  # queries per core = 8
KT = 128  # key tile size
EXP_GROUP = 2  # batches per exp instruction

F16 = mybir.dt.float16
F32 = mybir.dt.float32


def _build_graph(order, tiles, vls):
    """order: batch processing order; tiles[b]: valid 128-key tiles; vls[b]: valid_lens."""
    ntiles = [int(tiles[b]) for b in order]
    nvalid = [int(vls[b]) for b in order]
    T = int(sum(ntiles))
    # offs[i] = first task index of the i-th processed batch
    offs = np.concatenate([[0], np.cumsum(ntiles)]).astype(int)

    nc = bacc.Bacc("TRN2", target_bir_lowering=False, debug=False, num_devices=N_CORES)
    # per-batch contiguous packs, partition-major: kT[i] is [128, nt_i*256]
    kT_d = nc.dram_tensor("kT", [KT, T * 2 * KT], F16, kind="ExternalInput").ap()
    v1_d = nc.dram_tensor("v1", [KT, T * (DV + 1)], F16, kind="ExternalInput").ap()
    qT_d = nc.dram_tensor("qT", [2, KT, B * QS], F16, kind="ExternalInput").ap()
    wq_d = nc.dram_tensor("wqc", [2, KT, H], F16, kind="ExternalInput").ap()
    wk_d = nc.dram_tensor("wkc", [2, KT, H], F16, kind="ExternalInput").ap()
    wv_d = nc.dram_tensor("wv", [H, 1], F16, kind="ExternalInput").ap()
    out_d = nc.dram_tensor("out", [B * QS, DV], F32, kind="ExternalOutput").ap()

    with tile.TileContext(nc) as tc, ExitStack() as ctx:
        const = ctx.enter_context(tc.tile_pool(name="const", bufs=1))
        spool = ctx.enter_context(tc.tile_pool(name="s_psum", bufs=1, space="PSUM"))
        epool = ctx.enter_context(tc.tile_pool(name="e_sb", bufs=2))
        vpool = ctx.enter_context(tc.tile_pool(name="vals", bufs=4))
        fin = ctx.enter_context(tc.tile_pool(name="fin", bufs=1))
        opool = ctx.enter_context(tc.tile_pool(name="o_psum", bufs=2, space="PSUM"))

        # constants
        wq_sb = [const.tile([KT, H], F16, tag=f"wq{c}", name=f"wq{c}") for c in range(2)]
        wk_sb = [const.tile([KT, H], F16, tag=f"wk{c}", name=f"wk{c}") for c in range(2)]
        qT_sb = [
            const.tile([KT, B * QS], F16, tag=f"qt{c}", name=f"qt{c}") for c in range(2)
        ]
        for c in range(2):
            nc.sync.dma_start(wq_sb[c][:], wq_d[c])
            nc.sync.dma_start(wk_sb[c][:], wk_d[c])
            nc.sync.dma_start(qT_sb[c][:], qT_d[c])
        wv_sb = const.tile([H, 1], F16, tag="wv", name="wv_sb")
        nc.sync.dma_start(wv_sb[:], wv_d[:])
        qproj_sb = const.tile([H, B * QS], F32, tag="qproj", name="qproj_sb")

        # transposed scores: partition = key-in-tile, free = task*QS + j
        S = spool.tile([KT, T * QS], F32)
        # partial-tile columns are only written up to their valid rows;
        # zero once so exp() of the stale rows is finite (killed by V=0)
        nc.vector.memset(S[:, :], 0.0)

        # ---- phase 0: query projection ----
        with tc.tile_pool(name="qp_psum", bufs=1, space="PSUM") as qpp:
            qp = qpp.tile([H, B * QS], F32)
            for c in range(2):
                nc.tensor.matmul(
                    qp[:], wq_sb[c][:], qT_sb[c][:], start=(c == 0), stop=(c == 1)
                )
            nc.vector.tensor_copy(qproj_sb[:], qp[:])

        vals_sb = {}  # processed-batch index -> [128, nt*(DV+1)] tile

        with (
            tc.tile_pool(name="kt_sb", bufs=3) as ktp,
            tc.tile_pool(name="kp_psum", bufs=2, space="PSUM") as kpp,
            tc.tile_pool(name="kproj_sb", bufs=3) as kjp,
            tc.tile_pool(name="feat", bufs=8) as fp,
        ):
            kprojs = {}

            def emit_kproj(i):
                nt = ntiles[i]
                vlpad = nt * KT
                o = int(offs[i])
                # one contiguous DMA for the whole batch's keys^T and [V|1]
                vt = vpool.tile([KT, 8 * (DV + 1)], F16, tag="v1", name="vt")
                nc.sync.dma_start(
                    vt[:, : nt * (DV + 1)],
                    v1_d[:, o * (DV + 1) : (o + nt) * (DV + 1)],
                )
                vals_sb[i] = vt
                kt = ktp.tile([KT, 8 * 2 * KT], F16, tag="kt", name="kt")
                nc.sync.dma_start(
                    kt[:, : nt * 2 * KT], kT_d[:, o * 2 * KT : (o + nt) * 2 * KT]
                )
                vl_i = nvalid[i]
                kp = kpp.tile([H, KLEN], F32, tag="kp", name="kp")
                for t in range(nt):
                    w = min(KT, vl_i - t * KT)
                    sl = kp[:, t * KT : t * KT + w]
                    k0 = kt[:, (2 * t) * KT : (2 * t) * KT + w]
                    k1 = kt[:, (2 * t + 1) * KT : (2 * t + 1) * KT + w]
                    nc.tensor.matmul(sl, wk_sb[0][:], k0, start=True, stop=False)
                    nc.tensor.matmul(sl, wk_sb[1][:], k1, start=False, stop=True)
                kproj = kjp.tile([H, KLEN], F16, tag="kproj", name="kproj")
                nc.vector.tensor_copy(kproj[:, :vl_i], kp[:, :vl_i])
                kprojs[i] = kproj

            def emit_exp_and_av(i0, i1):
                # exp over the column range of processed batches [i0, i1)
                c0, c1 = int(offs[i0]) * QS, int(offs[i1]) * QS
                e = epool.tile([KT, EXP_GROUP * 8 * QS], F16, tag="e", name="e", bufs=3)
                nc.scalar.activation(
                    e[:, : c1 - c0], S[:, c0:c1], mybir.ActivationFunctionType.Exp
                )
                for i in range(i0, i1):
                    b = order[i]
                    nt = ntiles[i]
                    Ob = opool.tile([QS, DV + 1], F32, tag="ob", name="Ob")
                    for t in range(nt):
                        ec = (int(offs[i]) + t) * QS - c0
                        nc.tensor.matmul(
                            Ob[:],
                            e[:, ec : ec + QS],
                            vals_sb[i][:, t * (DV + 1) : (t + 1) * (DV + 1)],
                            start=(t == 0),
                            stop=(t == nt - 1),
                        )
                    vals_sb[i] = None
                    recip = fin.tile([QS, 1], F32, tag="recip", name="recip", bufs=2)
                    nc.vector.reciprocal(recip[:], Ob[:, DV : DV + 1])
                    outf = fin.tile([QS, DV], F32, tag="outf", name="outf", bufs=3)
                    nc.vector.tensor_scalar_mul(outf[:], Ob[:, :DV], recip[:])
                    nc.sync.dma_start(out_d[b * QS : (b + 1) * QS, :], outf[:])

            # kproj is computed one batch ahead so PE's in-order stream can
            # project batch i+1 while ACT runs batch i's tanh instructions.
            emit_kproj(0)
            for i in range(B):
                if i + 1 < B:
                    emit_kproj(i + 1)
                nt = ntiles[i]
                vl_i = nvalid[i]
                kproj = kprojs.pop(i)
                for j in range(QS):
                    bq = order[i] * QS + j
                    feat = fp.tile([H, KLEN], F16, tag="feat", name="feat")
                    nc.scalar.activation(
                        feat[:, :vl_i],
                        kproj[:, :vl_i],
                        mybir.ActivationFunctionType.Tanh,
                        bias=qproj_sb[:, bq : bq + 1],
                    )
                    for t in range(nt):
                        w = min(KT, vl_i - t * KT)
                        gcol = (int(offs[i]) + t) * QS + j
                        nc.tensor.matmul(
                            S[:w, gcol : gcol + 1],
                            feat[:, t * KT : t * KT + w],
                            wv_sb[:],
                            start=True,
                            stop=True,
                        )
                if i % EXP_GROUP == EXP_GROUP - 1:
                    emit_exp_and_av(i - EXP_GROUP + 1, i + 1)

    nc.compile()
    return nc


def kernel(queries, keys, values, valid_lens, Wq, Wk, wv):
    queries = np.asarray(queries, dtype=np.float32)
    keys = np.asarray(keys, dtype=np.float32)
    values = np.asarray(values, dtype=np.float32)
    vl = np.asarray(valid_lens).astype(np.int64)
    Wq = np.asarray(Wq, dtype=np.float32)
    Wk = np.asarray(Wk, dtype=np.float32)
    wv = np.asarray(wv, dtype=np.float32)

    tiles = np.maximum(1, -(-vl // KT))  # ceil, >=1
    asc = np.argsort(tiles, kind="stable")
    # three smallest batches first (fast ramp, light DMA warmup flood), then
    # descending so the last processed batches (the exp/AV tail) are small
    order = np.concatenate([asc[:3], asc[3:][::-1]])
    T = int(tiles.sum())

    # packed keys^T, partition-major: column block per task of [2*128] (d-chunks)
    kT_pack = np.empty((KT, T, 2, KT), dtype=np.float16)
    v1_pack = np.empty((KT, T, DV + 1), dtype=np.float16)
    v1_pack[:, :, DV] = 1.0
    i = 0
    for i_proc in range(B):
        b = int(order[i_proc])
        nt = int(tiles[b])
        for t in range(nt):
            ksl = keys[b, t * KT : (t + 1) * KT, :]  # [128 k, 256 d]
            # kT_pack[p, i, c, :] = keys[b, t*KT + (c*128..), p]^T chunks
            kT_pack[:, i, 0, :] = ksl[:, :KT].T
            kT_pack[:, i, 1, :] = ksl[:, KT:].T
            v1_pack[:, i, :DV] = values[b, t * KT : (t + 1) * KT, :]
            if t == nt - 1:
                rows = int(vl[b]) - t * KT
                v1_pack[rows:, i, :] = 0.0  # mask invalid keys via V and ones col
            i += 1
    kT_pack = kT_pack.reshape(KT, T * 2 * KT)
    v1_pack = v1_pack.reshape(KT, T * (DV + 1))

    wqc = Wq.reshape(2, KT, H).astype(np.float16)
    wkc = Wk.reshape(2, KT, H).astype(np.float16)
    wv_c = wv.reshape(H, 1).astype(np.float16)

    nc = _build_graph(order, tiles, vl)

    in_maps = []
    for c in range(N_CORES):
        qc = queries[:, c * QS : (c + 1) * QS, :].reshape(B * QS, D)
        qT = np.ascontiguousarray(qc.T).reshape(2, KT, B * QS).astype(np.float16)
        in_maps.append(
            {
                "kT": kT_pack,
                "v1": v1_pack,
                "qT": qT,
                "wqc": wqc,
                "wkc": wkc,
                "wv": wv_c,
            }
        )

    res = run_bass_kernel_spmd(nc, in_maps, core_ids=list(range(N_CORES)))

    out = np.empty((B, Q, DV), dtype=np.float32)
    for c in range(N_CORES):
        out[:, c * QS : (c + 1) * QS, :] = res.results[c]["out"].reshape(B, QS, DV)
    return out
